# revision 4
# baseline (speedup 1.0000x reference)
"""Trainium2 Bass kernel for nn_Attention_89670327206161.

Dense transformer attention, B=8 S=4096 D=1024 H=16 (dh=64), fp32.
The reference contracts attention scores over the *sequence* axis, so the
whole block collapses into weight-space:

    G        = x^T x                                  (D x D Gram matrix)
    scores_h = Wk_h G Wq_h^T / sqrt(dh)               (dh x dh per head)
    P_h      = softmax(scores_h, axis=-1)
    M        = sum_h Wv_h^T P_h Wo_h^T                (D x D)
    out      = x M

This needs ~21.7 GFLOP/core vs ~35.5 GFLOP for the direct Q/K/V path.
Sharding: pure data parallelism over batch, one element per core.

Per core:
  - x arrives seq-major; 32 resident s-tiles of [128, 1024].
  - G accumulates in PSUM in two passes (cols 512:1024 all row-chunks,
    then cols 0:512 for row-chunks 0..3); the bottom-left quadrant is
    mirrored from the top-right by PE transposes (G is symmetric).
  - Per 2-head-pair slab: Ct = G wkT-slab, scores = Ct^T-contract wqT-slab
    (256-wide rhs keeps fp32r at full rate), per-head 64x64 softmax into
    block-diagonal bf16 P tiles.
  - V/O path in bf16 (errors here don't touch the softmax; rel err ~2.6e-3
    vs 2e-2 budget): T1_h = P_h^T Wv_h, M += T1_h^T-contract woT rows.
  - out = x M per s-tile: 8 PE transposes rebuild the feature-major tile,
    then 16 fp32r matmuls of N=512.
"""

import numpy as np

HEADS = 16
B, S, D = 8, 4096, 1024
P = 128                  # partitions
NSC = S // P             # 32 seq tiles
NIC = D // P             # 8 feature chunks
NPAIR = HEADS // 2       # 8 head pairs (128-wide blocks)
N_CORES = 8

_PROGRAM = None


def _ts(i, n):
    return slice(i * n, (i + 1) * n)


def _build_program():
    import concourse.bacc as bacc
    import concourse.mybir as mybir
    import concourse.tile as tile

    f32 = mybir.dt.float32
    f32r = mybir.dt.float32r
    bf16 = mybir.dt.bfloat16
    EXP = mybir.ActivationFunctionType.Exp
    X = mybir.AxisListType.X

    nc = bacc.Bacc(trn_type="TRN2", debug=False, num_devices=N_CORES)

    xs_d = nc.dram_tensor("xs", [S, D], f32r, kind="ExternalInput")
    wkT_d = nc.dram_tensor("wkT", [D, D], f32r, kind="ExternalInput")
    wqT_d = nc.dram_tensor("wqT", [D, D], f32r, kind="ExternalInput")
    wv_d = nc.dram_tensor("wv", [D, D], bf16, kind="ExternalInput")
    wo_d = nc.dram_tensor("woT", [D, D], bf16, kind="ExternalInput")
    id_d = nc.dram_tensor("ident", [P, P], f32r, kind="ExternalInput")
    out_d = nc.dram_tensor("out", [S, D], f32, kind="ExternalOutput")

    wkTr = wkT_d.ap().rearrange("(c p) r -> p c r", p=P)   # [128, 8, 1024]
    wqTr = wqT_d.ap().rearrange("(c p) r -> p c r", p=P)
    wor = wo_d.ap().rearrange("(c p) o -> p c o", p=P)

    with tile.TileContext(nc) as tc:
      with (
          tc.tile_pool(name="xs", bufs=1) as xs_pool,
          tc.tile_pool(name="const", bufs=1) as const_pool,
      ):
        zero_sb = const_pool.tile([P, 512], f32r)
        nc.vector.memset(zero_sb[:].bitcast(f32), 0.0)
        ident_sb = const_pool.tile([P, P], f32r)
        nc.gpsimd.dma_start(ident_sb[:], id_d.ap())

        # resident seq-major x tiles; two HWDGE queues split the stream
        xs_t = []
        for t in range(NSC):
            xt_ = xs_pool.tile([P, D], f32r, tag=f"xs{t}")
            eng = nc.sync if t % 2 == 0 else nc.scalar
            eng.dma_start(xt_[:], xs_d.ap()[_ts(t, P), :])
            xs_t.append(xt_)

        with (
            tc.tile_pool(name="pb", bufs=1) as pb_pool,
            tc.tile_pool(name="t1", bufs=1) as t1_pool,
        ):
            pb_sb = []
            for hh in range(NPAIR):
                pbt = pb_pool.tile([P, P], bf16, tag=f"pb{hh}")
                nc.vector.memset(pbt[:].bitcast(f32), 0.0)
                pb_sb.append(pbt)

            with tc.tile_pool(name="gsb", bufs=1) as g_pool:
                g_sb = []
                for ic in range(NIC):
                    gt = g_pool.tile([P, D], f32r, tag=f"g{ic}")
                    g_sb.append(gt)

                # ---- G = x^T x ----
                with tc.tile_pool(name="gps", bufs=1, space="PSUM") as g_ps_pool:
                    gp = []
                    for ic in range(NIC):
                        gpt = g_ps_pool.tile([P, 512], f32, tag=f"gp{ic}")
                        gp.append(gpt)
                    # HAM warm-up: zero matmuls double as PSUM has_written
                    # clears for the pass-1 banks (idempotent, start=True)
                    for _ in range(2):
                        for ic in range(NIC):
                            nc.tensor.matmul(
                                gp[ic][:], zero_sb[:, 0:P], zero_sb[:],
                                start=True, stop=False, skip_group_check=True,
                            )
                    # pass 1: cols 512:1024, all row chunks
                    for t in range(NSC):
                        for ic in range(NIC):
                            nc.tensor.matmul(
                                gp[ic][:], xs_t[t][:, _ts(ic, P)],
                                xs_t[t][:, 512:D],
                                start=False, stop=(t == NSC - 1),
                                skip_group_check=True,
                            )
                    for ic in range(NIC):
                        eng = (nc.vector.tensor_copy if ic % 2 == 0
                               else nc.scalar.copy)
                        eng(g_sb[ic][:, 512:D], gp[ic][:])
                    # pass 2: cols 0:512, row chunks 0..3 (rest is mirrored)
                    gp2 = []
                    for ic in range(4):
                        gpt = g_ps_pool.tile([P, 512], f32, tag=f"gp{ic}")
                        gp2.append(gpt)
                    for t in range(NSC):
                        for ic in range(4):
                            nc.tensor.matmul(
                                gp2[ic][:], xs_t[t][:, _ts(ic, P)],
                                xs_t[t][:, 0:512],
                                start=(t == 0), stop=(t == NSC - 1),
                                skip_group_check=True,
                            )
                    for ic in range(4):
                        eng = (nc.vector.tensor_copy if ic % 2 == 0
                               else nc.scalar.copy)
                        eng(g_sb[ic][:, 0:512], gp2[ic][:])

                # mirror bottom-left from top-right (G symmetric)
                with tc.tile_pool(name="mir", bufs=4, space="PSUM") as mir_pool:
                    k = 0
                    for ic in range(4):
                        for jc2 in range(4):
                            mp = mir_pool.tile([P, P], f32r, tag="mir")
                            nc.tensor.transpose(
                                mp[:],
                                g_sb[ic][:, 512 + jc2 * P:512 + (jc2 + 1) * P],
                                ident_sb[:],
                            )
                            eng = (nc.vector.tensor_copy if k % 2 == 0
                                   else nc.scalar.copy)
                            eng(g_sb[4 + jc2][:, _ts(ic, P)], mp[:])
                            k += 1

                # ---- scores + softmax per 2-pair slab ----
                with (
                    tc.tile_pool(name="wk", bufs=1) as wk_pool,
                    tc.tile_pool(name="wq", bufs=1) as wq_pool,
                    tc.tile_pool(name="ct", bufs=1) as ct_pool,
                    tc.tile_pool(name="ctps", bufs=1, space="PSUM") as ct_ps_pool,
                    tc.tile_pool(name="scps", bufs=2, space="PSUM") as sc_ps_pool,
                    tc.tile_pool(name="smx", bufs=4) as smx_pool,
                ):
                    for hh2 in range(4):
                        wk_sb = wk_pool.tile([P, NIC, 256], f32r, tag="wk")
                        nc.gpsimd.dma_start(wk_sb[:], wkTr[:, :, _ts(hh2, 256)])
                        wq_sb = wq_pool.tile([P, NIC, 256], f32r, tag="wq")
                        nc.gpsimd.dma_start(wq_sb[:], wqTr[:, :, _ts(hh2, 256)])

                        ct_ps = ct_ps_pool.tile([P, NIC, 256], f32, tag="ctps")
                        for jc in range(NIC):
                            for ic in range(NIC):
                                nc.tensor.matmul(
                                    ct_ps[:, jc, :], g_sb[ic][:, _ts(jc, P)],
                                    wk_sb[:, ic, :],
                                    start=(ic == 0), stop=(ic == NIC - 1),
                                )
                        ct_sb = ct_pool.tile([P, NIC, 256], f32r, tag="ct")
                        nc.vector.tensor_copy(ct_sb[:, 0:4, :], ct_ps[:, 0:4, :])
                        nc.scalar.copy(ct_sb[:, 4:8, :], ct_ps[:, 4:8, :])

                        for pr in range(2):
                            hh = hh2 * 2 + pr
                            sc_ps = sc_ps_pool.tile([P, 256], f32, tag="scps")
                            for jc in range(NIC):
                                nc.tensor.matmul(
                                    sc_ps[:], ct_sb[:, jc, _ts(pr, P)],
                                    wq_sb[:, jc, :],
                                    start=(jc == 0), stop=(jc == NIC - 1),
                                )
                            # per-head softmax on the diagonal 64x64 blocks
                            for hf in range(2):
                                rows = slice(64 * hf, 64 * hf + 64)
                                cols = slice(pr * P + 64 * hf,
                                             pr * P + 64 * hf + 64)
                                pcols = slice(64 * hf, 64 * hf + 64)
                                mx = smx_pool.tile([P, 1], f32, tag="mx")
                                nmx = smx_pool.tile([P, 1], f32, tag="nmx")
                                nc.vector.reduce_max(
                                    mx[rows, 0:1], sc_ps[rows, cols], axis=X,
                                    negate=True,
                                )
                                nc.vector.tensor_scalar_mul(
                                    nmx[rows, 0:1], mx[rows, 0:1], 0.125
                                )
                                p_tmp = smx_pool.tile([P, 64], f32, tag="ptmp")
                                nc.scalar.activation(
                                    p_tmp[rows, :], sc_ps[rows, cols], EXP,
                                    bias=nmx[rows, 0:1], scale=0.125,
                                )
                                den = smx_pool.tile([P, 1], f32, tag="den")
                                rec = smx_pool.tile([P, 1], f32, tag="rec")
                                nc.vector.reduce_sum(
                                    den[rows, 0:1], p_tmp[rows, :], axis=X
                                )
                                nc.vector.reciprocal(
                                    rec[rows, 0:1], den[rows, 0:1]
                                )
                                nc.vector.tensor_scalar_mul(
                                    pb_sb[hh][rows, pcols], p_tmp[rows, :],
                                    rec[rows, 0:1],
                                )

            # ---- T1_h = P_h^T Wv_h (g_sb freed above) ----
            t1_sb = []
            with (
                tc.tile_pool(name="wv", bufs=2) as wv_pool,
                tc.tile_pool(name="t1ps", bufs=2, space="PSUM") as t1_ps_pool,
            ):
                for hh in range(NPAIR):
                    wv_sb = wv_pool.tile([P, D], bf16, tag="wv")
                    nc.gpsimd.dma_start(wv_sb[:], wv_d.ap()[_ts(hh, P), :])
                    t1_ps = t1_ps_pool.tile([P, D], f32, tag="t1ps")
                    for half in range(2):
                        nc.tensor.matmul(
                            t1_ps[:, _ts(half, 512)], pb_sb[hh][:],
                            wv_sb[:, _ts(half, 512)],
                            start=True, stop=True,
                        )
                    t1t = t1_pool.tile([P, D], bf16, tag=f"t1{hh}")
                    nc.vector.tensor_copy(t1t[:, 0:512], t1_ps[:, 0:512])
                    nc.scalar.copy(t1t[:, 512:D], t1_ps[:, 512:D])
                    t1_sb.append(t1t)

            # ---- M = sum_h T1_h^T-contract woT rows; then out = x M ----
            with tc.tile_pool(name="m", bufs=1) as m_pool:
                m_sb = []
                for ic in range(NIC):
                    mt = m_pool.tile([P, D], f32r, tag=f"m{ic}")
                    m_sb.append(mt)

                with (
                    tc.tile_pool(name="wo", bufs=1) as wo_pool,
                    tc.tile_pool(name="mps", bufs=1, space="PSUM") as m_ps_pool,
                ):
                    for half in range(2):
                        wo_sb = wo_pool.tile([P, NIC, 512], bf16, tag="wo")
                        nc.gpsimd.dma_start(wo_sb[:], wor[:, :, _ts(half, 512)])
                        mp = []
                        for ic in range(NIC):
                            mpt = m_ps_pool.tile([P, 512], f32, tag=f"mp{ic}")
                            mp.append(mpt)
                        for hh in range(NPAIR):
                            for ic in range(NIC):
                                nc.tensor.matmul(
                                    mp[ic][:], t1_sb[hh][:, _ts(ic, P)],
                                    wo_sb[:, hh, :],
                                    start=(hh == 0), stop=(hh == NPAIR - 1),
                                )
                        for ic in range(NIC):
                            eng = (nc.vector.tensor_copy if ic % 2 == 0
                                   else nc.scalar.copy)
                            eng(m_sb[ic][:, _ts(half, 512)], mp[ic][:])

                # ---- out = x M per s-tile ----
                with (
                    tc.tile_pool(name="xt", bufs=2) as xt_pool,
                    tc.tile_pool(name="ob", bufs=3) as ob_pool,
                    tc.tile_pool(name="trps", bufs=2, space="PSUM") as tr_ps_pool,
                    tc.tile_pool(name="ops", bufs=4, space="PSUM") as out_ps_pool,
                ):
                    for t in range(NSC):
                        tr_ps = tr_ps_pool.tile([P, NIC, P], f32r, tag="trps")
                        for ic in range(NIC):
                            nc.tensor.transpose(
                                tr_ps[:, ic, :], xs_t[t][:, _ts(ic, P)],
                                ident_sb[:],
                            )
                        xt_sb = xt_pool.tile([P, NIC, P], f32r, tag="xt")
                        nc.vector.tensor_copy(xt_sb[:, 0:4, :], tr_ps[:, 0:4, :])
                        nc.scalar.copy(xt_sb[:, 4:8, :], tr_ps[:, 4:8, :])

                        ob = ob_pool.tile([P, D], f32, tag="ob")
                        for half in range(2):
                            op = out_ps_pool.tile([P, 512], f32, tag="ops")
                            for ic in range(NIC):
                                nc.tensor.matmul(
                                    op[:], xt_sb[:, ic, :],
                                    m_sb[ic][:, _ts(half, 512)],
                                    start=(ic == 0), stop=(ic == NIC - 1),
                                )
                            eng = (nc.vector.tensor_copy if half == 0
                                   else nc.scalar.copy)
                            eng(ob[:, _ts(half, 512)], op[:])
                        eng = nc.sync if t % 2 == 0 else nc.scalar
                        eng.dma_start(out_d.ap()[_ts(t, P), :], ob[:])

    nc.compile()
    return nc


def _get_program():
    global _PROGRAM
    if _PROGRAM is None:
        _PROGRAM = _build_program()
    return _PROGRAM


def kernel(x, Wq, Wk, Wv, Wo):
    import ml_dtypes
    from concourse import bass_utils

    nc = _get_program()

    x = np.asarray(x, np.float32)
    wkT = np.ascontiguousarray(np.asarray(Wk, np.float32).T)
    wqT = np.ascontiguousarray(np.asarray(Wq, np.float32).T)
    wv_b = np.asarray(Wv, np.float32).astype(ml_dtypes.bfloat16)
    wo_b = np.ascontiguousarray(np.asarray(Wo, np.float32).T).astype(
        ml_dtypes.bfloat16
    )
    ident = np.eye(P, dtype=np.float32)

    in_maps = [
        {"xs": np.ascontiguousarray(x[b]), "wkT": wkT, "wqT": wqT,
         "wv": wv_b, "woT": wo_b, "ident": ident}
        for b in range(N_CORES)
    ]
    res = bass_utils.run_bass_kernel_spmd(nc, in_maps, core_ids=list(range(N_CORES)))
    return np.stack([res.results[b]["out"] for b in range(N_CORES)], axis=0)


# revision 27
# speedup vs baseline: 2.2997x; 2.2997x over previous
"""Trainium2 Bass kernel for nn_Attention_89670327206161.

Dense transformer attention, B=8 S=4096 D=1024 H=16 (dh=64), fp32.
The reference contracts attention scores over the *sequence* axis, so the
whole block collapses into weight-space:

    G        = x^T x                                  (D x D Gram matrix)
    scores_h = Wk_h G Wq_h^T / sqrt(dh)               (dh x dh per head)
    P_h      = softmax(scores_h, axis=-1)
    M        = sum_h Wv_h^T P_h Wo_h^T                (D x D)
    out      = x M

This needs ~21.7 GFLOP/core vs ~35.5 GFLOP for the direct Q/K/V path.
Sharding: pure data parallelism over batch, one element per core.

Per core:
  - x arrives seq-major in f32r; 32 resident s-tiles feed G only and are
    freed afterwards.
  - G accumulates in PSUM in two passes; lower-triangle 128-blocks that
    keep the matmul >=256 wide are skipped and mirrored by PE transposes
    (G is symmetric).
  - Per 2-head-pair slab: Ct = G wkT-slab, scores = Ct^T-contract
    wqT-slab (256-wide rhs keeps fp32r at full rate), per-head 64x64
    softmax into block-diagonal bf16 P tiles.
  - V/O path and the output matmul run in bf16 (errors here don't touch
    the softmax; rel err ~3.3e-3 vs the 2e-2 budget): T1_h = P_h^T Wv_h,
    M += T1_h^T-contract woT rows, out = x M with a host-supplied
    feature-major bf16 copy of x (no on-chip transposes).
"""

import numpy as np

HEADS = 16
B, S, D = 8, 4096, 1024
P = 128                  # partitions
NSC = S // P             # 32 seq tiles
NIC = D // P             # 8 feature chunks
NPAIR = HEADS // 2       # 8 head pairs (128-wide blocks)
N_CORES = 8

_PROGRAM = None


def _ts(i, n):
    return slice(i * n, (i + 1) * n)


def _build_program(reps=1):
    import concourse.bacc as bacc
    import concourse.mybir as mybir
    import concourse.tile as tile

    f32 = mybir.dt.float32
    f32r = mybir.dt.float32r
    bf16 = mybir.dt.bfloat16
    EXP = mybir.ActivationFunctionType.Exp
    X = mybir.AxisListType.X

    nc = bacc.Bacc(trn_type="TRN2", debug=False, num_devices=N_CORES)

    xs_d = nc.dram_tensor("xs", [S, D], f32r, kind="ExternalInput")
    xtb_d = nc.dram_tensor("xTb", [D, S], bf16, kind="ExternalInput")
    wkT_d = nc.dram_tensor("wkT", [D, D], f32r, kind="ExternalInput")
    wqT_d = nc.dram_tensor("wqT", [D, D], f32r, kind="ExternalInput")
    wv_d = nc.dram_tensor("wv", [D, D], bf16, kind="ExternalInput")
    wo_d = nc.dram_tensor("woT", [D, D], bf16, kind="ExternalInput")
    id_d = nc.dram_tensor("ident", [P, P], f32r, kind="ExternalInput")
    out_d = nc.dram_tensor("out", [S, D], f32, kind="ExternalOutput")

    wkTr = wkT_d.ap().rearrange("(c p) r -> p c r", p=P)   # [128, 8, 1024]
    wqTr = wqT_d.ap().rearrange("(c p) r -> p c r", p=P)
    wor = wo_d.ap().rearrange("(c p) o -> p c o", p=P)
    xtbr = xtb_d.ap().rearrange("(c p) s -> p c s", p=P)   # [128, 8, 4096]

    with tile.TileContext(nc) as tc:
     for _rep in range(reps):
      with tc.tile_pool(name="const", bufs=1) as const_pool:
        zero_sb = const_pool.tile([P, 512], f32r)
        nc.vector.memset(zero_sb[:].bitcast(f32), 0.0)
        ident_sb = const_pool.tile([P, P], f32r)
        nc.gpsimd.dma_start(ident_sb[:], id_d.ap())

        with (
            tc.tile_pool(name="pb", bufs=1) as pb_pool,
            tc.tile_pool(name="t1", bufs=1) as t1_pool,
        ):
            pb_sb = []
            for hh in range(NPAIR):
                pbt = pb_pool.tile([P, P], bf16, tag=f"pb{hh}")
                nc.vector.memset(pbt[:].bitcast(f32), 0.0)
                pb_sb.append(pbt)

            with tc.tile_pool(name="gsb", bufs=1) as g_pool:
                g_sb = []
                for ic in range(NIC):
                    gt = g_pool.tile([P, D], f32r, tag=f"g{ic}")
                    g_sb.append(gt)

                # ---- G = x^T x (x resident only for this phase) ----
                with (
                    tc.tile_pool(name="xs", bufs=1) as xs_pool,
                    tc.tile_pool(name="gps", bufs=1, space="PSUM") as g_ps_pool,
                ):
                    # seq-major x tiles; two HWDGE queues split the stream
                    xs_t = []
                    for t in range(NSC):
                        xt_ = xs_pool.tile([P, D], f32r, tag=f"xs{t}")
                        eng = nc.sync if t % 2 == 0 else nc.scalar
                        eng.dma_start(xt_[:], xs_d.ap()[_ts(t, P), :])
                        xs_t.append(xt_)

                    gp = []
                    for ic in range(NIC):
                        gpt = g_ps_pool.tile([P, 512], f32, tag=f"gp{ic}")
                        gp.append(gpt)
                    # HAM warm-up: zero matmuls double as PSUM has_written
                    # clears for the pass-1 banks (idempotent, start=True)
                    for _ in range(2):
                        for ic in range(NIC):
                            nc.tensor.matmul(
                                gp[ic][:], zero_sb[:, 0:P], zero_sb[:],
                                start=True, stop=False, skip_group_check=True,
                            )
                    # pass 1: cols 512:1024, all row chunks. Row chunks 5,6
                    # skip their in-quadrant lower-triangle blocks (mirrored
                    # later); chunk 7's would leave N=128 (quarter-rate
                    # fp32r), so it computes the full width.
                    p1_off = {0: 0, 1: 0, 2: 0, 3: 0, 4: 0, 5: 128, 6: 256, 7: 0}
                    for t in range(NSC):
                        for ic in range(NIC):
                            off = p1_off[ic]
                            nc.tensor.matmul(
                                gp[ic][:, off:512], xs_t[t][:, _ts(ic, P)],
                                xs_t[t][:, 512 + off:D],
                                start=False, stop=(t == NSC - 1),
                                skip_group_check=True,
                            )
                    # flush the slots pass 2 reuses (ic 0..3) on the fast DVE
                    for ic in range(NIC):
                        eng = (nc.vector.tensor_copy if ic < 4
                               else nc.scalar.copy)
                        eng(g_sb[ic][:, 512:D], gp[ic][:])
                    # pass 2: cols 0:512, row chunks 0..3 (rest is mirrored)
                    gp2 = []
                    for ic in range(4):
                        gpt = g_ps_pool.tile([P, 512], f32, tag=f"gp{ic}")
                        gp2.append(gpt)
                    p2_off = {0: 0, 1: 128, 2: 256, 3: 0}
                    for t in range(NSC):
                        for ic in range(4):
                            off = p2_off[ic]
                            nc.tensor.matmul(
                                gp2[ic][:, off:512], xs_t[t][:, _ts(ic, P)],
                                xs_t[t][:, off:512],
                                start=(t == 0), stop=(t == NSC - 1),
                                skip_group_check=True,
                            )
                    for ic in range(4):
                        eng = (nc.vector.tensor_copy if ic % 2 == 0
                               else nc.scalar.copy)
                        eng(g_sb[ic][:, 0:512], gp2[ic][:])

                # mirror the bottom-left quadrant plus the skipped
                # in-quadrant lower-triangle blocks (G symmetric);
                # (r, c) = destination block row/col in 128-units
                mirrors = [(4 + jc2, ic) for ic in range(4) for jc2 in range(4)]
                mirrors += [(5, 4), (6, 4), (6, 5), (1, 0), (2, 0), (2, 1)]
                with tc.tile_pool(name="mir", bufs=4, space="PSUM") as mir_pool:
                    for k, (r, c) in enumerate(mirrors):
                        mp = mir_pool.tile([P, P], f32r, tag="mir")
                        nc.tensor.transpose(
                            mp[:], g_sb[c][:, _ts(r, P)], ident_sb[:]
                        )
                        eng = (nc.vector.tensor_copy if k % 2 == 0
                               else nc.scalar.copy)
                        eng(g_sb[r][:, _ts(c, P)], mp[:])

                # ---- scores + softmax per 2-pair slab ----
                with (
                    tc.tile_pool(name="wk", bufs=2) as wk_pool,
                    tc.tile_pool(name="wq", bufs=2) as wq_pool,
                    tc.tile_pool(name="ct", bufs=2) as ct_pool,
                    tc.tile_pool(name="ctps", bufs=1, space="PSUM") as ct_ps_pool,
                    tc.tile_pool(name="scps", bufs=2, space="PSUM") as sc_ps_pool,
                    tc.tile_pool(name="smx", bufs=4) as smx_pool,
                ):
                    for hh2 in range(4):
                        wk_sb = wk_pool.tile([P, NIC, 256], f32r, tag="wk")
                        nc.sync.dma_start(wk_sb[:], wkTr[:, :, _ts(hh2, 256)])
                        wq_sb = wq_pool.tile([P, NIC, 256], f32r, tag="wq")
                        nc.scalar.dma_start(wq_sb[:], wqTr[:, :, _ts(hh2, 256)])

                        ct_ps = ct_ps_pool.tile([P, NIC, 256], f32, tag="ctps")
                        ct_sb = ct_pool.tile([P, NIC, 256], f32r, tag="ct")
                        for jc in range(NIC):
                            for ic in range(NIC):
                                nc.tensor.matmul(
                                    ct_ps[:, jc, :], g_sb[ic][:, _ts(jc, P)],
                                    wk_sb[:, ic, :],
                                    start=(ic == 0), stop=(ic == NIC - 1),
                                )
                        # column-split: pair-0's lhsT columns land via DVE
                        # first so its scores aren't stuck behind ACT
                        nc.vector.tensor_copy(ct_sb[:, :, 0:P], ct_ps[:, :, 0:P])
                        nc.scalar.copy(ct_sb[:, :, P:256], ct_ps[:, :, P:256])

                        for pr in range(2):
                            hh = hh2 * 2 + pr
                            sc_ps = sc_ps_pool.tile([P, 256], f32, tag="scps")
                            for jc in range(NIC):
                                nc.tensor.matmul(
                                    sc_ps[:], ct_sb[:, jc, _ts(pr, P)],
                                    wq_sb[:, jc, :],
                                    start=(jc == 0), stop=(jc == NIC - 1),
                                )
                            # per-head softmax on the diagonal 64x64 blocks
                            for hf in range(2):
                                rows = slice(64 * hf, 64 * hf + 64)
                                cols = slice(pr * P + 64 * hf,
                                             pr * P + 64 * hf + 64)
                                pcols = slice(64 * hf, 64 * hf + 64)
                                mx = smx_pool.tile([P, 1], f32, tag="mx")
                                nmx = smx_pool.tile([P, 1], f32, tag="nmx")
                                nc.vector.reduce_max(
                                    mx[rows, 0:1], sc_ps[rows, cols], axis=X,
                                    negate=True,
                                )
                                nc.vector.tensor_scalar_mul(
                                    nmx[rows, 0:1], mx[rows, 0:1], 0.125
                                )
                                p_tmp = smx_pool.tile([P, 64], f32, tag="ptmp")
                                nc.scalar.activation(
                                    p_tmp[rows, :], sc_ps[rows, cols], EXP,
                                    bias=nmx[rows, 0:1], scale=0.125,
                                )
                                den = smx_pool.tile([P, 1], f32, tag="den")
                                rec = smx_pool.tile([P, 1], f32, tag="rec")
                                nc.vector.reduce_sum(
                                    den[rows, 0:1], p_tmp[rows, :], axis=X
                                )
                                nc.vector.reciprocal(
                                    rec[rows, 0:1], den[rows, 0:1]
                                )
                                nc.vector.tensor_scalar_mul(
                                    pb_sb[hh][rows, pcols], p_tmp[rows, :],
                                    rec[rows, 0:1],
                                )

            # ---- T1_h = P_h^T Wv_h (g_sb freed above) ----
            t1_sb = []
            with (
                tc.tile_pool(name="wv", bufs=1) as wv_pool,
                tc.tile_pool(name="t1ps", bufs=2, space="PSUM") as t1_ps_pool,
            ):
                # two 1 MB wv transfers on the idle SWDGE queue start early
                wvr = wv_d.ap().rearrange("(c p) i -> p c i", p=P)
                wv_lo = wv_pool.tile([P, 4, D], bf16, tag="wv0")
                nc.gpsimd.dma_start(wv_lo[:], wvr[:, 0:4, :])
                wv_hi = wv_pool.tile([P, 4, D], bf16, tag="wv1")
                nc.gpsimd.dma_start(wv_hi[:], wvr[:, 4:8, :])
                for hh in range(NPAIR):
                    wv_sb = (wv_lo if hh < 4 else wv_hi)[:, hh % 4, :]
                    t1_ps = t1_ps_pool.tile([P, D], f32, tag="t1ps")
                    for half in range(2):
                        nc.tensor.matmul(
                            t1_ps[:, _ts(half, 512)], pb_sb[hh][:],
                            wv_sb[:, _ts(half, 512)],
                            start=True, stop=True,
                        )
                    t1t = t1_pool.tile([P, D], bf16, tag=f"t1{hh}")
                    nc.vector.tensor_copy(t1t[:, 0:512], t1_ps[:, 0:512])
                    nc.scalar.copy(t1t[:, 512:D], t1_ps[:, 512:D])
                    t1_sb.append(t1t)

            # ---- M = sum_h T1_h^T-contract woT rows; then out = x M ----
            with tc.tile_pool(name="m", bufs=1) as m_pool:
                m_sb = []
                for ic in range(NIC):
                    mt = m_pool.tile([P, D], bf16, tag=f"m{ic}")
                    m_sb.append(mt)

                with (
                    tc.tile_pool(name="wo", bufs=2) as wo_pool,
                    tc.tile_pool(name="mps", bufs=1, space="PSUM") as m_ps_pool,
                ):
                    # quarter-column wo slabs, double-buffered on the scalar
                    # queue so M accumulation never waits on a weight DMA
                    for q in range(4):
                        wo_sb = wo_pool.tile([P, NIC, 256], bf16, tag="wo")
                        nc.scalar.dma_start(wo_sb[:], wor[:, :, _ts(q, 256)])
                        mp = []
                        for ic in range(NIC):
                            mpt = m_ps_pool.tile([P, 256], f32, tag=f"mp{ic}")
                            mp.append(mpt)
                        for hh in range(NPAIR):
                            for ic in range(NIC):
                                nc.tensor.matmul(
                                    mp[ic][:], t1_sb[hh][:, _ts(ic, P)],
                                    wo_sb[:, hh, :],
                                    start=(hh == 0), stop=(hh == NPAIR - 1),
                                )
                        for ic in range(NIC):
                            eng = (nc.vector.tensor_copy if ic % 2 == 0
                                   else nc.scalar.copy)
                            eng(m_sb[ic][:, _ts(q, 256)], mp[ic][:])

                # ---- out = x M per 512-seq block, bf16 x from host ----
                with (
                    tc.tile_pool(name="xtb", bufs=2) as xtb_pool,
                    tc.tile_pool(name="ob", bufs=3) as ob_pool,
                    tc.tile_pool(name="ops", bufs=4, space="PSUM") as out_ps_pool,
                ):
                    NSB = 8  # 512-wide seq blocks
                    for sb in range(NSB):
                        xtb_sb = xtb_pool.tile([P, NIC, 512], bf16, tag="xtb")
                        eng = nc.sync if sb % 2 == 0 else nc.scalar
                        eng.dma_start(xtb_sb[:], xtbr[:, :, _ts(sb, 512)])
                        for st in range(4):
                            t = sb * 4 + st
                            ob = ob_pool.tile([P, D], f32, tag="ob")
                            for half in range(2):
                                op = out_ps_pool.tile([P, 512], f32, tag="ops")
                                for ic in range(NIC):
                                    nc.tensor.matmul(
                                        op[:],
                                        xtb_sb[:, ic, _ts(st, P)],
                                        m_sb[ic][:, _ts(half, 512)],
                                        start=(ic == 0), stop=(ic == NIC - 1),
                                    )
                                eng = (nc.vector.tensor_copy if half == 0
                                       else nc.scalar.copy)
                                eng(ob[:, _ts(half, 512)], op[:])
                            eng = nc.sync if t % 2 == 0 else nc.scalar
                            eng.dma_start(out_d.ap()[_ts(t, P), :], ob[:])

    nc.compile()
    return nc


def _get_program():
    global _PROGRAM
    if _PROGRAM is None:
        _PROGRAM = _build_program()
    return _PROGRAM


def kernel(x, Wq, Wk, Wv, Wo):
    import ml_dtypes
    from concourse import bass_utils

    nc = _get_program()

    x = np.asarray(x, np.float32)
    wkT = np.ascontiguousarray(np.asarray(Wk, np.float32).T)
    wqT = np.ascontiguousarray(np.asarray(Wq, np.float32).T)
    wv_b = np.asarray(Wv, np.float32).astype(ml_dtypes.bfloat16)
    wo_b = np.ascontiguousarray(np.asarray(Wo, np.float32).T).astype(
        ml_dtypes.bfloat16
    )
    ident = np.eye(P, dtype=np.float32)

    in_maps = []
    for b in range(N_CORES):
        xb = np.ascontiguousarray(x[b])
        xtb = np.ascontiguousarray(xb.T).astype(ml_dtypes.bfloat16)
        in_maps.append(
            {"xs": xb, "xTb": xtb, "wkT": wkT, "wqT": wqT,
             "wv": wv_b, "woT": wo_b, "ident": ident}
        )
    res = bass_utils.run_bass_kernel_spmd(nc, in_maps, core_ids=list(range(N_CORES)))
    return np.stack([res.results[b]["out"] for b in range(N_CORES)], axis=0)


# revision 32
# speedup vs baseline: 3.5091x; 1.5259x over previous
"""Trainium2 Bass kernel for nn_Attention_89670327206161.

Dense transformer attention, B=8 S=4096 D=1024 H=16 (dh=64), fp32.
The reference contracts attention scores over the *sequence* axis, so the
whole block collapses into weight-space:

    G        = x^T x                                  (D x D Gram matrix)
    scores_h = Wk_h G Wq_h^T / sqrt(dh)               (dh x dh per head)
    P_h      = softmax(scores_h, axis=-1)
    M        = sum_h Wv_h^T P_h Wo_h^T                (D x D)
    out      = x M

This needs ~21.7 GFLOP/core vs ~35.5 GFLOP for the direct Q/K/V path.
Sharding: pure data parallelism over batch, one element per core.

Per core:
  - x arrives seq-major in f32r; 32 resident s-tiles feed G only and are
    freed afterwards.
  - G accumulates in PSUM in two passes; lower-triangle 128-blocks that
    keep the matmul >=256 wide are skipped and mirrored by PE transposes
    (G is symmetric).
  - Per 2-head-pair slab: Ct = G wkT-slab, scores = Ct^T-contract
    wqT-slab (256-wide rhs keeps fp32r at full rate), per-head 64x64
    softmax into block-diagonal bf16 P tiles.
  - V/O path and the output matmul run in bf16 (errors here don't touch
    the softmax; rel err ~3.3e-3 vs the 2e-2 budget): T1_h = P_h^T Wv_h,
    M += T1_h^T-contract woT rows, out = x M with a host-supplied
    feature-major bf16 copy of x (no on-chip transposes).
"""

import numpy as np

HEADS = 16
B, S, D = 8, 4096, 1024
P = 128                  # partitions
NSC = S // P             # 32 seq tiles
NIC = D // P             # 8 feature chunks
NPAIR = HEADS // 2       # 8 head pairs (128-wide blocks)
N_CORES = 8

_PROGRAM = None


def _ts(i, n):
    return slice(i * n, (i + 1) * n)


def _build_program(reps=1):
    import concourse.bacc as bacc
    import concourse.mybir as mybir
    import concourse.tile as tile

    f32 = mybir.dt.float32
    f32r = mybir.dt.float32r
    bf16 = mybir.dt.bfloat16
    EXP = mybir.ActivationFunctionType.Exp
    X = mybir.AxisListType.X

    nc = bacc.Bacc(trn_type="TRN2", debug=False, num_devices=N_CORES)

    xs_d = nc.dram_tensor("xs", [S, D], f32r, kind="ExternalInput")
    xtb_d = nc.dram_tensor("xTb", [D, S], bf16, kind="ExternalInput")
    wkT_d = nc.dram_tensor("wkT", [D, D], f32r, kind="ExternalInput")
    wqT_d = nc.dram_tensor("wqT", [D, D], f32r, kind="ExternalInput")
    wv_d = nc.dram_tensor("wv", [D, D], bf16, kind="ExternalInput")
    wo_d = nc.dram_tensor("woT", [D, D], bf16, kind="ExternalInput")
    id_d = nc.dram_tensor("ident", [P, P], f32r, kind="ExternalInput")
    out_d = nc.dram_tensor("out", [S, D], f32, kind="ExternalOutput")

    wkTr = wkT_d.ap().rearrange("(c p) r -> p c r", p=P)   # [128, 8, 1024]
    wqTr = wqT_d.ap().rearrange("(c p) r -> p c r", p=P)
    wor = wo_d.ap().rearrange("(c p) o -> p c o", p=P)
    xtbr = xtb_d.ap().rearrange("(c p) s -> p c s", p=P)   # [128, 8, 4096]

    with tile.TileContext(nc) as tc:
     for _rep in range(reps):
      with tc.tile_pool(name="const", bufs=1) as const_pool:
        zero_sb = const_pool.tile([P, 512], f32r)
        nc.vector.memset(zero_sb[:].bitcast(f32), 0.0)
        ident_sb = const_pool.tile([P, P], f32r)
        nc.gpsimd.dma_start(ident_sb[:], id_d.ap())

        with (
            tc.tile_pool(name="pb", bufs=1) as pb_pool,
            tc.tile_pool(name="t1", bufs=1) as t1_pool,
        ):
            pb_sb = []
            for hh in range(NPAIR):
                pbt = pb_pool.tile([P, P], bf16, tag=f"pb{hh}")
                nc.vector.memset(pbt[:].bitcast(f32), 0.0)
                pb_sb.append(pbt)

            with tc.tile_pool(name="gsb", bufs=1) as g_pool:
                g_sb = []
                for ic in range(NIC):
                    gt = g_pool.tile([P, D], f32r, tag=f"g{ic}")
                    g_sb.append(gt)

                # ---- G = x^T x (x resident only for this phase) ----
                with (
                    tc.tile_pool(name="xs", bufs=1) as xs_pool,
                    tc.tile_pool(name="gps", bufs=1, space="PSUM") as g_ps_pool,
                ):
                    # seq-major x tiles; two HWDGE queues split the stream
                    xs_t = []
                    for t in range(NSC):
                        xt_ = xs_pool.tile([P, D], f32r, tag=f"xs{t}")
                        eng = nc.sync if t % 2 == 0 else nc.scalar
                        eng.dma_start(xt_[:], xs_d.ap()[_ts(t, P), :])
                        xs_t.append(xt_)

                    gp = []
                    for ic in range(NIC):
                        gpt = g_ps_pool.tile([P, 512], f32, tag=f"gp{ic}")
                        gp.append(gpt)
                    # HAM warm-up: zero matmuls double as PSUM has_written
                    # clears for the pass-1 banks (idempotent, start=True)
                    for _ in range(2):
                        for ic in range(NIC):
                            nc.tensor.matmul(
                                gp[ic][:], zero_sb[:, 0:P], zero_sb[:],
                                start=True, stop=False, skip_group_check=True,
                            )
                    # pass 1: cols 512:1024, all row chunks. Row chunks 5,6
                    # skip their in-quadrant lower-triangle blocks (mirrored
                    # later); chunk 7's would leave N=128 (quarter-rate
                    # fp32r), so it computes the full width.
                    p1_off = {0: 0, 1: 0, 2: 0, 3: 0, 4: 0, 5: 128, 6: 256, 7: 0}
                    for t in range(NSC):
                        for ic in range(NIC):
                            off = p1_off[ic]
                            nc.tensor.matmul(
                                gp[ic][:, off:512], xs_t[t][:, _ts(ic, P)],
                                xs_t[t][:, 512 + off:D],
                                start=False, stop=(t == NSC - 1),
                                skip_group_check=True,
                            )
                    # flush the slots pass 2 reuses (ic 0..3) on the fast DVE
                    for ic in range(NIC):
                        eng = (nc.vector.tensor_copy if ic < 4
                               else nc.scalar.copy)
                        eng(g_sb[ic][:, 512:D], gp[ic][:])
                    # pass 2: cols 0:512, row chunks 0..3 (rest is mirrored)
                    gp2 = []
                    for ic in range(4):
                        gpt = g_ps_pool.tile([P, 512], f32, tag=f"gp{ic}")
                        gp2.append(gpt)
                    p2_off = {0: 0, 1: 128, 2: 256, 3: 0}
                    for t in range(NSC):
                        for ic in range(4):
                            off = p2_off[ic]
                            nc.tensor.matmul(
                                gp2[ic][:, off:512], xs_t[t][:, _ts(ic, P)],
                                xs_t[t][:, off:512],
                                start=(t == 0), stop=(t == NSC - 1),
                                skip_group_check=True,
                            )
                    for ic in range(4):
                        eng = (nc.vector.tensor_copy if ic % 2 == 0
                               else nc.scalar.copy)
                        eng(g_sb[ic][:, 0:512], gp2[ic][:])

                # mirror the bottom-left quadrant plus the skipped
                # in-quadrant lower-triangle blocks (G symmetric);
                # (r, c) = destination block row/col in 128-units
                mirrors = [(4 + jc2, ic) for ic in range(4) for jc2 in range(4)]
                mirrors += [(5, 4), (6, 4), (6, 5), (1, 0), (2, 0), (2, 1)]
                with tc.tile_pool(name="mir", bufs=4, space="PSUM") as mir_pool:
                    for k, (r, c) in enumerate(mirrors):
                        mp = mir_pool.tile([P, P], f32r, tag="mir")
                        nc.tensor.transpose(
                            mp[:], g_sb[c][:, _ts(r, P)], ident_sb[:]
                        )
                        eng = (nc.vector.tensor_copy if k % 2 == 0
                               else nc.scalar.copy)
                        eng(g_sb[r][:, _ts(c, P)], mp[:])

                # ---- scores + softmax per 2-pair slab; T1 for the previous
                # slab's pairs is interleaved so the PE never idles on
                # softmax or PSUM release ----
                t1_sb = []
                with (
                    tc.tile_pool(name="wk", bufs=2) as wk_pool,
                    tc.tile_pool(name="wq", bufs=2) as wq_pool,
                    tc.tile_pool(name="ct", bufs=2) as ct_pool,
                    tc.tile_pool(name="wv", bufs=1) as wv_pool,
                    tc.tile_pool(name="ctps", bufs=1, space="PSUM") as ct_ps_pool,
                    tc.tile_pool(name="scps", bufs=2, space="PSUM") as sc_ps_pool,
                    tc.tile_pool(name="t1ps", bufs=1, space="PSUM") as t1_ps_pool,
                    tc.tile_pool(name="smx", bufs=4) as smx_pool,
                ):
                    # two 1 MB wv transfers on the idle SWDGE queue start early
                    wvr = wv_d.ap().rearrange("(c p) i -> p c i", p=P)
                    wv_lo = wv_pool.tile([P, 4, D], bf16, tag="wv0")
                    nc.gpsimd.dma_start(wv_lo[:], wvr[:, 0:4, :])
                    wv_hi = wv_pool.tile([P, 4, D], bf16, tag="wv1")
                    nc.gpsimd.dma_start(wv_hi[:], wvr[:, 4:8, :])

                    def emit_t1(hh):
                        wv_sb = (wv_lo if hh < 4 else wv_hi)[:, hh % 4, :]
                        t1_ps = t1_ps_pool.tile([P, D], f32, tag="t1ps")
                        for half in range(2):
                            nc.tensor.matmul(
                                t1_ps[:, _ts(half, 512)], pb_sb[hh][:],
                                wv_sb[:, _ts(half, 512)],
                                start=True, stop=True,
                            )
                        t1t = t1_pool.tile([P, D], bf16, tag=f"t1{hh}")
                        nc.vector.tensor_copy(t1t[:, 0:512], t1_ps[:, 0:512])
                        nc.scalar.copy(t1t[:, 512:D], t1_ps[:, 512:D])
                        t1_sb.append(t1t)

                    for hh2 in range(4):
                        wk_sb = wk_pool.tile([P, NIC, 256], f32r, tag="wk")
                        nc.sync.dma_start(wk_sb[:], wkTr[:, :, _ts(hh2, 256)])
                        wq_sb = wq_pool.tile([P, NIC, 256], f32r, tag="wq")
                        nc.scalar.dma_start(wq_sb[:], wqTr[:, :, _ts(hh2, 256)])

                        ct_ps = ct_ps_pool.tile([P, NIC, 256], f32, tag="ctps")
                        ct_sb = ct_pool.tile([P, NIC, 256], f32r, tag="ct")
                        for jc in range(NIC):
                            for ic in range(NIC):
                                nc.tensor.matmul(
                                    ct_ps[:, jc, :], g_sb[ic][:, _ts(jc, P)],
                                    wk_sb[:, ic, :],
                                    start=(ic == 0), stop=(ic == NIC - 1),
                                )
                        # column-split: pair-0's lhsT columns land via DVE
                        # first so its scores aren't stuck behind ACT
                        nc.vector.tensor_copy(ct_sb[:, :, 0:P], ct_ps[:, :, 0:P])
                        nc.scalar.copy(ct_sb[:, :, P:256], ct_ps[:, :, P:256])

                        for pr in range(2):
                            hh = hh2 * 2 + pr
                            sc_ps = sc_ps_pool.tile([P, 256], f32, tag="scps")
                            for jc in range(NIC):
                                nc.tensor.matmul(
                                    sc_ps[:], ct_sb[:, jc, _ts(pr, P)],
                                    wq_sb[:, jc, :],
                                    start=(jc == 0), stop=(jc == NIC - 1),
                                )
                            if pr == 1 and hh2 > 0:
                                emit_t1(hh2 * 2 - 2)
                                emit_t1(hh2 * 2 - 1)
                            # per-head softmax on the diagonal 64x64 blocks
                            for hf in range(2):
                                rows = slice(64 * hf, 64 * hf + 64)
                                cols = slice(pr * P + 64 * hf,
                                             pr * P + 64 * hf + 64)
                                pcols = slice(64 * hf, 64 * hf + 64)
                                mx = smx_pool.tile([P, 1], f32, tag="mx")
                                nmx = smx_pool.tile([P, 1], f32, tag="nmx")
                                nc.vector.reduce_max(
                                    mx[rows, 0:1], sc_ps[rows, cols], axis=X,
                                    negate=True,
                                )
                                nc.vector.tensor_scalar_mul(
                                    nmx[rows, 0:1], mx[rows, 0:1], 0.125
                                )
                                p_tmp = smx_pool.tile([P, 64], f32, tag="ptmp")
                                nc.scalar.activation(
                                    p_tmp[rows, :], sc_ps[rows, cols], EXP,
                                    bias=nmx[rows, 0:1], scale=0.125,
                                )
                                den = smx_pool.tile([P, 1], f32, tag="den")
                                rec = smx_pool.tile([P, 1], f32, tag="rec")
                                nc.vector.reduce_sum(
                                    den[rows, 0:1], p_tmp[rows, :], axis=X
                                )
                                nc.vector.reciprocal(
                                    rec[rows, 0:1], den[rows, 0:1]
                                )
                                nc.vector.tensor_scalar_mul(
                                    pb_sb[hh][rows, pcols], p_tmp[rows, :],
                                    rec[rows, 0:1],
                                )
                    # last slab's pairs
                    emit_t1(6)
                    emit_t1(7)

            # ---- M = sum_h T1_h^T-contract woT rows; then out = x M ----
            with (
                tc.tile_pool(name="m", bufs=1) as m_pool,
                tc.tile_pool(name="xtb", bufs=2) as xtb_pool,
            ):
                m_sb = []
                for ic in range(NIC):
                    mt = m_pool.tile([P, D], bf16, tag=f"m{ic}")
                    m_sb.append(mt)

                # prefetch the first two bf16 x^T slabs during M so the
                # output matmuls start the moment M lands
                NSB = 8  # 512-wide seq blocks
                xtb_tiles = []

                def emit_xtb(sb):
                    xtb_sb = xtb_pool.tile([P, NIC, 512], bf16, tag="xtb")
                    eng = nc.sync if sb % 2 == 0 else nc.scalar
                    eng.dma_start(xtb_sb[:], xtbr[:, :, _ts(sb, 512)])
                    xtb_tiles.append(xtb_sb)

                emit_xtb(0)
                emit_xtb(1)

                with (
                    tc.tile_pool(name="wo", bufs=2) as wo_pool,
                    tc.tile_pool(name="mps", bufs=1, space="PSUM") as m_ps_pool,
                ):
                    # quarter-column wo slabs, double-buffered on the scalar
                    # queue so M accumulation never waits on a weight DMA
                    for q in range(4):
                        wo_sb = wo_pool.tile([P, NIC, 256], bf16, tag="wo")
                        nc.scalar.dma_start(wo_sb[:], wor[:, :, _ts(q, 256)])
                        mp = []
                        for ic in range(NIC):
                            mpt = m_ps_pool.tile([P, 256], f32, tag=f"mp{ic}")
                            mp.append(mpt)
                        for hh in range(NPAIR):
                            for ic in range(NIC):
                                nc.tensor.matmul(
                                    mp[ic][:], t1_sb[hh][:, _ts(ic, P)],
                                    wo_sb[:, hh, :],
                                    start=(hh == 0), stop=(hh == NPAIR - 1),
                                )
                        for ic in range(NIC):
                            eng = (nc.vector.tensor_copy if ic % 2 == 0
                                   else nc.scalar.copy)
                            eng(m_sb[ic][:, _ts(q, 256)], mp[ic][:])

                # ---- out = x M per 512-seq block, bf16 x from host ----
                with (
                    tc.tile_pool(name="ob", bufs=3) as ob_pool,
                    tc.tile_pool(name="ops", bufs=4, space="PSUM") as out_ps_pool,
                ):
                    for sb in range(NSB):
                        if sb + 2 < NSB:
                            emit_xtb(sb + 2)
                        xtb_sb = xtb_tiles[sb]
                        for st in range(4):
                            t = sb * 4 + st
                            ob = ob_pool.tile([P, D], f32, tag="ob")
                            for half in range(2):
                                op = out_ps_pool.tile([P, 512], f32, tag="ops")
                                for ic in range(NIC):
                                    nc.tensor.matmul(
                                        op[:],
                                        xtb_sb[:, ic, _ts(st, P)],
                                        m_sb[ic][:, _ts(half, 512)],
                                        start=(ic == 0), stop=(ic == NIC - 1),
                                    )
                                eng = (nc.vector.tensor_copy if half == 0
                                       else nc.scalar.copy)
                                eng(ob[:, _ts(half, 512)], op[:])
                            eng = nc.sync if t % 2 == 0 else nc.scalar
                            eng.dma_start(out_d.ap()[_ts(t, P), :], ob[:])

    nc.compile()
    return nc


def _get_program():
    global _PROGRAM
    if _PROGRAM is None:
        _PROGRAM = _build_program()
    return _PROGRAM


def kernel(x, Wq, Wk, Wv, Wo):
    import ml_dtypes
    from concourse import bass_utils

    nc = _get_program()

    x = np.asarray(x, np.float32)
    wkT = np.ascontiguousarray(np.asarray(Wk, np.float32).T)
    wqT = np.ascontiguousarray(np.asarray(Wq, np.float32).T)
    wv_b = np.asarray(Wv, np.float32).astype(ml_dtypes.bfloat16)
    wo_b = np.ascontiguousarray(np.asarray(Wo, np.float32).T).astype(
        ml_dtypes.bfloat16
    )
    ident = np.eye(P, dtype=np.float32)

    in_maps = []
    for b in range(N_CORES):
        xb = np.ascontiguousarray(x[b])
        xtb = np.ascontiguousarray(xb.T).astype(ml_dtypes.bfloat16)
        in_maps.append(
            {"xs": xb, "xTb": xtb, "wkT": wkT, "wqT": wqT,
             "wv": wv_b, "woT": wo_b, "ident": ident}
        )
    res = bass_utils.run_bass_kernel_spmd(nc, in_maps, core_ids=list(range(N_CORES)))
    return np.stack([res.results[b]["out"] for b in range(N_CORES)], axis=0)
